# revision 23
# baseline (speedup 1.0000x reference)
"""MOLELinear (mixture-of-linear-experts) Trainium2 kernel, v4.

Math (per group g): out_g = x_g @ (sum_e c[g,e] W_e + W_sh).T + (sum_e c[g,e] b_e + b_sh)

Sharding: data-parallel over the 32 groups -> 4 groups (8192 tokens) per core,
expert weights replicated. Host does layout-only prep (transpose / stacking /
dtype staging to bf16 -- the device math runs in bf16 regardless); all FLOPs
(weight mixing, bias mixing, GEMM, bias add) run on device.

Engine plan (per core):
  - DMA ~21.5 MB bf16 (x 8.4 + W 4.7 + out 8.4). DMA *issue* is split across
    the two HWDGE engines (SP + ACT) because each dma_start costs ~0.8us of
    issue time on its engine; w pieces + x(g0) land by ~10us.
  - DVE: all weight mixing as 4 full-FD chains: tensor_scalar mult (4x packed)
    + tensor_tensor add (2x packed) over contiguous [128, 2048] e-major
    slices, ~16us per group. scalar_tensor_tensor is never used (it has no
    DVE accel modes); GpSimd is never used for elementwise (its Q7 SBUF
    traffic slows concurrent DVE ops ~10x).
  - PE: 256 bf16 matmuls, ts-inner so 4 consecutive matmuls share the same
    stationary weight tile.
  - ScalarE(ACT): PSUM drains with the mixed bias fused as a per-partition
    AP (transposed orientation makes the bias per-partition).

Weight HBM layout: 9 pieces [128, 2048] bf16 (piece 0 = shared, first to
arrive; piece 1+j = expert j), inner [kt(4)][o(512)]. Output written
transposed [512 o, 8192 t] bf16; host transposes/upcasts.
"""
import ml_dtypes
import numpy as np

import concourse.bacc as bacc
import concourse.mybir as mybir
from concourse.alu_op_type import AluOpType
from concourse.tile import TileContext
from concourse.bass_utils import run_bass_kernel_spmd

N_CORES = 8
IN_F = 512
OUT_F = 512
N_EXPERTS = 8
N_GROUPS = 32
TOK_PER_GROUP = 2048
G_PER_CORE = N_GROUPS // N_CORES           # 4
TOK_PER_CORE = G_PER_CORE * TOK_PER_GROUP  # 8192
KT = IN_F // 128                           # 4 k-tiles
OT = OUT_F // 128                          # 4 o-tiles
WSZ = KT * OUT_F                           # 2048 = one expert's k-major slice
F32 = mybir.dt.float32
F32R = mybir.dt.float32r
BF16 = mybir.dt.bfloat16

_CACHE = {}


def _build():
    nc = bacc.Bacc(trn_type="TRN2")
    # x: [g*KT+kt][p][t]  (k = kt*128 + p)
    x = nc.dram_tensor("x", (G_PER_CORE * KT, 128, TOK_PER_GROUP), BF16, kind="ExternalInput")
    # w: [piece(9)][p][kt(4)*o(512)]; piece 0 = shared, 1+j = expert j
    wt = nc.dram_tensor("wt", (N_EXPERTS + 1, 128, WSZ), BF16, kind="ExternalInput")
    cb = nc.dram_tensor("cb", (128, G_PER_CORE * N_EXPERTS), F32, kind="ExternalInput")
    cx = nc.dram_tensor("cx", (N_EXPERTS + 1, G_PER_CORE), F32R, kind="ExternalInput")
    ball = nc.dram_tensor("ball", (N_EXPERTS + 1, OUT_F), F32R, kind="ExternalInput")
    out = nc.dram_tensor("out", (OUT_F, TOK_PER_CORE), BF16, kind="ExternalOutput")

    with TileContext(nc) as tc:
        with (
            tc.tile_pool(name="wp", bufs=1) as wp,
            tc.tile_pool(name="mixp", bufs=1) as mixp,
            tc.tile_pool(name="smallp", bufs=1) as smallp,
            tc.tile_pool(name="xp", bufs=3) as xp,
            tc.tile_pool(name="op", bufs=4) as op,
            tc.tile_pool(name="psp", bufs=7, space="PSUM") as psp,
            tc.tile_pool(name="psb", bufs=1, space="PSUM") as psb,
        ):
            # ---- DMA issue split across the two HWDGE engines (SP + ACT), ----
            # ---- chain-critical pieces (w0, w1, cb) first ----
            wall = wp.tile([128, (N_EXPERTS + 1) * WSZ], BF16, tag="wall")
            xgs = []
            for g in range(G_PER_CORE):
                xgs.append(xp.tile([128, KT * TOK_PER_GROUP], BF16, tag="x", name=f"xg{g}"))
            cbt = smallp.tile([128, G_PER_CORE * N_EXPERTS], F32, tag="cb")
            cxt = smallp.tile([N_EXPERTS + 1, G_PER_CORE], F32R, tag="cx")
            ballt = smallp.tile([N_EXPERTS + 1, OUT_F], F32R, tag="ball")

            def wdma(eng, e):
                eng.dma_start(wall[:, e * WSZ : (e + 1) * WSZ], wt[e])

            wdma(nc.sync, 0)
            wdma(nc.scalar, 1)
            nc.sync.dma_start(cbt[:], cb[:])
            wdma(nc.scalar, 2)
            wdma(nc.sync, 3)
            wdma(nc.scalar, 4)
            wdma(nc.sync, 5)
            wdma(nc.scalar, 6)
            wdma(nc.sync, 7)
            wdma(nc.scalar, 8)
            nc.sync.dma_start(cxt[:], cx[:])
            nc.sync.dma_start(ballt[:], ball[:])
            # x for group 0 strictly after all w pieces: w must win the
            # bandwidth race (mixing gates everything; x(g0) isn't needed
            # until the first chain completes)
            for kt in range(KT):
                nc.scalar.dma_start(
                    xgs[0][:, kt * TOK_PER_GROUP : (kt + 1) * TOK_PER_GROUP], x[kt]
                )

            def wsl(e):
                return wall[:, e * WSZ : (e + 1) * WSZ]

            # ---- mixed biases, transposed: mb[o, g] = ball.T @ cx ----
            psmb = psb.tile([128, 512], F32, tag="psmb")
            for ot in range(OT):
                nc.tensor.matmul(
                    psmb[:, ot * G_PER_CORE : (ot + 1) * G_PER_CORE],
                    ballt[:, ot * 128 : (ot + 1) * 128],
                    cxt[:],
                    start=True,
                    stop=True,
                )
            mbv = smallp.tile([128, OT * G_PER_CORE], F32, tag="mbv")
            nc.vector.tensor_copy(mbv[:], psmb[:, : OT * G_PER_CORE])

            # ---- weight mixing: all on DVE as TS (4x) mults + tree-structured
            # TT (2x) adds (same op count as a chain, but depth 4, so the mix
            # completes ~2 ops after the last w piece lands instead of 8).
            # groups 0-2: full-FD [128,2048]; group 3: two o-half trees so its
            # GEMM overlaps the end of mixing.
            def mix_tree(g, dst, wpiece, usz, utag):
                # dst += sum_e c[g,e]*wpiece(1+e) + wpiece(0); emission order
                # interleaves scales and adds to consume pieces as they arrive
                cbc = lambda e: cbt[:, g * N_EXPERTS + e - 1 : g * N_EXPERTS + e]
                u = {}

                def ts(e):
                    u[e] = mixp.tile([128, usz], BF16, tag=utag, bufs=8,
                                     name=f"u{utag}{g}_{e}")
                    nc.vector.tensor_scalar(u[e][:], wpiece(e), cbc(e), None,
                                            AluOpType.mult)

                def tt(o, a, b):
                    nc.vector.tensor_tensor(o, a, b, AluOpType.add)

                ts(1)
                tt(u[1][:], u[1][:], wpiece(0))       # a = c1*W1 + Wsh
                ts(2); ts(3)
                tt(u[2][:], u[2][:], u[3][:])         # b = u2 + u3
                ts(4); ts(5)
                tt(u[4][:], u[4][:], u[5][:])         # c = u4 + u5
                ts(6); ts(7)
                tt(u[6][:], u[6][:], u[7][:])         # d = u6 + u7
                tt(u[1][:], u[1][:], u[2][:])         # e = a + b
                ts(8)
                tt(u[4][:], u[4][:], u[6][:])         # f = c + d
                tt(u[1][:], u[1][:], u[4][:])         # g = e + f
                tt(dst, u[1][:], u[8][:])             # dst = g + u8

            wm = []
            for g in range(3):
                t = mixp.tile([128, WSZ], BF16, tag=f"wm{g}", name=f"wm{g}")
                mix_tree(g, t[:], wsl, WSZ, "u")
                wm.append(t)
            wm3 = mixp.tile([128, WSZ], BF16, tag="wm3", name="wm3")
            for oth in range(2):
                def whalf(e, oth=oth):
                    return wall[:, e * WSZ + oth * 1024 : e * WSZ + (oth + 1) * 1024]

                mix_tree(3, wm3[:, oth * 1024 : (oth + 1) * 1024], whalf, 1024,
                         f"uh{oth}")
            wm.append(wm3)

            # ---- main GEMM, transposed: psum[o, t] = wm.T @ x ----
            for g in range(G_PER_CORE):
                # prefetch next group's x before this group's out-DMAs can
                # block SP's issue FIFO on drain semaphores; hold each
                # prefetch back (modeled time) so it does not steal DMA
                # bandwidth from the startup-critical w + x(g0) transfers
                if g + 1 < G_PER_CORE:
                    with tc.tile_wait_until(0.016 + 0.011 * g):
                        for kt in range(KT):
                            nc.sync.dma_start(
                                xgs[g + 1][:, kt * TOK_PER_GROUP : (kt + 1) * TOK_PER_GROUP],
                                x[(g + 1) * KT + kt],
                            )
                xg = xgs[g]
                for ot in range(OT):
                    oc = op.tile([128, TOK_PER_GROUP], BF16, tag="oc", name=f"oc{g}_{ot}")
                    units = []
                    for ts in range(4):
                        units.append(psp.tile([128, 512], F32, tag="ps", name=f"ps{g}_{ot}_{ts}"))
                    for kt in range(KT):
                        off = (ot // 2) * 1024 + kt * 256 + (ot % 2) * 128
                        wslice = wm[g][:, off : off + 128]
                        for ts in range(4):
                            nc.tensor.matmul(
                                units[ts][:],
                                wslice,
                                xg[:, kt * TOK_PER_GROUP + ts * 512 : kt * TOK_PER_GROUP + (ts + 1) * 512],
                                start=(kt == 0),
                                stop=(kt == KT - 1),
                            )
                    last = g == G_PER_CORE - 1
                    for ts in range(4):
                        mcol = mbv[:, ot * G_PER_CORE + g : ot * G_PER_CORE + g + 1]
                        if last and ts >= 2:
                            # DVE is idle once mixing ends; split the final
                            # group's drains to shorten the kernel tail
                            nc.vector.tensor_scalar(
                                oc[:, ts * 512 : (ts + 1) * 512],
                                units[ts][:], mcol, None, AluOpType.add,
                            )
                        else:
                            nc.scalar.add(
                                oc[:, ts * 512 : (ts + 1) * 512],
                                units[ts][:], mcol,
                            )
                        if last:
                            # fine-grained out-DMA on the final group to
                            # shorten the kernel tail
                            t0 = g * TOK_PER_GROUP + ts * 512
                            nc.sync.dma_start(
                                out[ot * 128 : (ot + 1) * 128, t0 : t0 + 512],
                                oc[:, ts * 512 : (ts + 1) * 512],
                            )
                    if not last:
                        nc.sync.dma_start(
                            out[ot * 128 : (ot + 1) * 128, g * TOK_PER_GROUP : (g + 1) * TOK_PER_GROUP],
                            oc[:],
                        )
    nc.finalize()
    return nc


def kernel(x, coefficients, weight_experts, bias_experts, weight_shared, bias_shared, sizes):
    x = np.asarray(x)
    coefficients = np.asarray(coefficients)
    weight_experts = np.asarray(weight_experts)
    bias_experts = np.asarray(bias_experts)
    weight_shared = np.asarray(weight_shared)
    bias_shared = np.asarray(bias_shared)

    if "nc" not in _CACHE:
        _CACHE["nc"] = _build()
    nc = _CACHE["nc"]

    # ---- host-side layout prep ----
    bf16 = ml_dtypes.bfloat16
    wt9 = np.empty((N_EXPERTS + 1, IN_F, OUT_F), np.float32)
    wt9[0] = weight_shared.T
    for e in range(N_EXPERTS):
        wt9[1 + e] = weight_experts[e].T
    # [piece][kt][p][oth][o256] -> [piece][p][oth][kt][o256]
    wt_np = np.ascontiguousarray(
        wt9.reshape(N_EXPERTS + 1, KT, 128, 2, 256).transpose(0, 2, 3, 1, 4)
    ).astype(bf16).reshape(N_EXPERTS + 1, 128, WSZ)
    ball_np = np.empty((N_EXPERTS + 1, OUT_F), np.float32)
    ball_np[:N_EXPERTS] = bias_experts
    ball_np[N_EXPERTS] = bias_shared

    in_maps = []
    for c in range(N_CORES):
        gs = slice(c * G_PER_CORE, (c + 1) * G_PER_CORE)
        cg = coefficients[gs]  # [4, 8]
        cb_np = np.broadcast_to(
            cg.reshape(1, -1), (128, G_PER_CORE * N_EXPERTS)
        ).copy()
        cx_np = np.empty((N_EXPERTS + 1, G_PER_CORE), np.float32)
        cx_np[:N_EXPERTS] = cg.T
        cx_np[N_EXPERTS] = 1.0
        xs = x[c * TOK_PER_CORE : (c + 1) * TOK_PER_CORE]
        # [g][t][kt][p] -> [g][kt][p][t]
        x_np = np.ascontiguousarray(
            xs.reshape(G_PER_CORE, TOK_PER_GROUP, KT, 128).transpose(0, 2, 3, 1)
        ).astype(bf16)
        in_maps.append(
            {
                "x": x_np,
                "wt": wt_np,
                "cb": cb_np,
                "cx": cx_np,
                "ball": ball_np,
            }
        )

    res = run_bass_kernel_spmd(nc, in_maps, core_ids=list(range(N_CORES)))
    return np.concatenate(
        [res.results[c]["out"].astype(np.float32).T for c in range(N_CORES)], axis=0
    )


# revision 28
# speedup vs baseline: 1.0313x; 1.0313x over previous
"""MOLELinear (mixture-of-linear-experts) Trainium2 kernel, v4.

Math (per group g): out_g = x_g @ (sum_e c[g,e] W_e + W_sh).T + (sum_e c[g,e] b_e + b_sh)

Sharding: data-parallel over the 32 groups -> 4 groups (8192 tokens) per core,
expert weights replicated. Host does layout-only prep (transpose / stacking /
dtype staging to bf16 -- the device math runs in bf16 regardless); all FLOPs
(weight mixing, bias mixing, GEMM, bias add) run on device.

Engine plan (per core):
  - DMA ~21.5 MB bf16 (x 8.4 + W 4.7 + out 8.4). DMA *issue* is split across
    the two HWDGE engines (SP + ACT) because each dma_start costs ~0.8us of
    issue time on its engine; w pieces + x(g0) land by ~10us.
  - DVE: all weight mixing as 4 full-FD chains: tensor_scalar mult (4x packed)
    + tensor_tensor add (2x packed) over contiguous [128, 2048] e-major
    slices, ~16us per group. scalar_tensor_tensor is never used (it has no
    DVE accel modes); GpSimd is never used for elementwise (its Q7 SBUF
    traffic slows concurrent DVE ops ~10x).
  - PE: 256 bf16 matmuls, ts-inner so 4 consecutive matmuls share the same
    stationary weight tile.
  - ScalarE(ACT): PSUM drains with the mixed bias fused as a per-partition
    AP (transposed orientation makes the bias per-partition).

Weight HBM layout: 9 pieces [128, 2048] bf16 (piece 0 = shared, first to
arrive; piece 1+j = expert j), inner [kt(4)][o(512)]. Output written
transposed [512 o, 8192 t] bf16; host transposes/upcasts.
"""
import ml_dtypes
import numpy as np

import concourse.bacc as bacc
import concourse.mybir as mybir
from concourse.alu_op_type import AluOpType
from concourse.tile import TileContext
from concourse.bass_utils import run_bass_kernel_spmd

N_CORES = 8
IN_F = 512
OUT_F = 512
N_EXPERTS = 8
N_GROUPS = 32
TOK_PER_GROUP = 2048
G_PER_CORE = N_GROUPS // N_CORES           # 4
TOK_PER_CORE = G_PER_CORE * TOK_PER_GROUP  # 8192
KT = IN_F // 128                           # 4 k-tiles
OT = OUT_F // 128                          # 4 o-tiles
WSZ = KT * OUT_F                           # 2048 = one expert's k-major slice
F32 = mybir.dt.float32
F32R = mybir.dt.float32r
BF16 = mybir.dt.bfloat16

_CACHE = {}


def _build():
    nc = bacc.Bacc(trn_type="TRN2")
    # x: [g*4+ts][p][kt*512t]  (token-major pieces: one piece = 512 tokens,
    # all k; the first psum unit needs only one piece)
    x = nc.dram_tensor("x", (G_PER_CORE * KT, 128, TOK_PER_GROUP), BF16, kind="ExternalInput")
    # w: [piece(9)][p][kt(4)*o(512)]; piece 0 = shared, 1+j = expert j
    wt = nc.dram_tensor("wt", (N_EXPERTS + 1, 128, WSZ), BF16, kind="ExternalInput")
    cb = nc.dram_tensor("cb", (128, G_PER_CORE * N_EXPERTS), F32, kind="ExternalInput")
    cx = nc.dram_tensor("cx", (N_EXPERTS + 1, G_PER_CORE), F32R, kind="ExternalInput")
    ball = nc.dram_tensor("ball", (N_EXPERTS + 1, OUT_F), F32R, kind="ExternalInput")
    out = nc.dram_tensor("out", (OUT_F, TOK_PER_CORE), BF16, kind="ExternalOutput")

    with TileContext(nc) as tc:
        with (
            tc.tile_pool(name="wp", bufs=1) as wp,
            tc.tile_pool(name="mixp", bufs=1) as mixp,
            tc.tile_pool(name="smallp", bufs=1) as smallp,
            tc.tile_pool(name="xp", bufs=3) as xp,
            tc.tile_pool(name="op", bufs=4) as op,
            tc.tile_pool(name="psp", bufs=7, space="PSUM") as psp,
            tc.tile_pool(name="psb", bufs=1, space="PSUM") as psb,
        ):
            # ---- DMA issue split across the two HWDGE engines (SP + ACT), ----
            # ---- chain-critical pieces (w0, w1, cb) first ----
            wall = wp.tile([128, (N_EXPERTS + 1) * WSZ], BF16, tag="wall")
            xgs = []
            for g in range(G_PER_CORE):
                xgs.append(xp.tile([128, KT * TOK_PER_GROUP], BF16, tag="x", name=f"xg{g}"))
            cbt = smallp.tile([128, G_PER_CORE * N_EXPERTS], F32, tag="cb")
            cxt = smallp.tile([N_EXPERTS + 1, G_PER_CORE], F32R, tag="cx")
            ballt = smallp.tile([N_EXPERTS + 1, OUT_F], F32R, tag="ball")

            def wdma(eng, e):
                eng.dma_start(wall[:, e * WSZ : (e + 1) * WSZ], wt[e])

            wdma(nc.sync, 0)
            wdma(nc.scalar, 1)
            nc.sync.dma_start(cbt[:], cb[:])
            wdma(nc.scalar, 2)
            wdma(nc.sync, 3)
            wdma(nc.scalar, 4)
            wdma(nc.sync, 5)
            wdma(nc.scalar, 6)
            wdma(nc.sync, 7)
            wdma(nc.scalar, 8)
            nc.sync.dma_start(cxt[:], cx[:])
            nc.sync.dma_start(ballt[:], ball[:])
            # x for group 0 strictly after all w pieces: w must win the
            # bandwidth race (mixing gates everything; x(g0) isn't needed
            # until the first chain completes)
            for ts in range(KT):
                nc.scalar.dma_start(
                    xgs[0][:, ts * TOK_PER_GROUP : (ts + 1) * TOK_PER_GROUP], x[ts]
                )

            def wsl(e):
                return wall[:, e * WSZ : (e + 1) * WSZ]

            # ---- mixed biases, transposed: mb[o, g] = ball.T @ cx ----
            psmb = psb.tile([128, 512], F32, tag="psmb")
            for ot in range(OT):
                nc.tensor.matmul(
                    psmb[:, ot * G_PER_CORE : (ot + 1) * G_PER_CORE],
                    ballt[:, ot * 128 : (ot + 1) * 128],
                    cxt[:],
                    start=True,
                    stop=True,
                )
            mbv = smallp.tile([128, OT * G_PER_CORE], F32, tag="mbv")
            nc.vector.tensor_copy(mbv[:], psmb[:, : OT * G_PER_CORE])

            # ---- weight mixing: all on DVE as TS (4x) mults + tree-structured
            # TT (2x) adds (same op count as a chain, but depth 4, so the mix
            # completes ~2 ops after the last w piece lands instead of 8).
            # groups 0-2: full-FD [128,2048]; group 3: two o-half trees so its
            # GEMM overlaps the end of mixing.
            def mix_tree(g, dst, wpiece, usz, utag):
                # dst += sum_e c[g,e]*wpiece(1+e) + wpiece(0); emission order
                # interleaves scales and adds to consume pieces as they arrive
                cbc = lambda e: cbt[:, g * N_EXPERTS + e - 1 : g * N_EXPERTS + e]
                u = {}

                def ts(e):
                    u[e] = mixp.tile([128, usz], BF16, tag=utag, bufs=8,
                                     name=f"u{utag}{g}_{e}")
                    nc.vector.tensor_scalar(u[e][:], wpiece(e), cbc(e), None,
                                            AluOpType.mult)

                def tt(o, a, b):
                    nc.vector.tensor_tensor(o, a, b, AluOpType.add)

                ts(1)
                tt(u[1][:], u[1][:], wpiece(0))       # a = c1*W1 + Wsh
                ts(2); ts(3)
                tt(u[2][:], u[2][:], u[3][:])         # b = u2 + u3
                ts(4); ts(5)
                tt(u[4][:], u[4][:], u[5][:])         # c = u4 + u5
                ts(6); ts(7)
                tt(u[6][:], u[6][:], u[7][:])         # d = u6 + u7
                tt(u[1][:], u[1][:], u[2][:])         # e = a + b
                ts(8)
                tt(u[4][:], u[4][:], u[6][:])         # f = c + d
                tt(u[1][:], u[1][:], u[4][:])         # g = e + f
                tt(dst, u[1][:], u[8][:])             # dst = g + u8

            wm = []
            for g in range(3):
                t = mixp.tile([128, WSZ], BF16, tag=f"wm{g}", name=f"wm{g}")
                mix_tree(g, t[:], wsl, WSZ, "u")
                wm.append(t)
            wm3 = mixp.tile([128, WSZ], BF16, tag="wm3", name="wm3")
            for oth in range(2):
                def whalf(e, oth=oth):
                    return wall[:, e * WSZ + oth * 1024 : e * WSZ + (oth + 1) * 1024]

                mix_tree(3, wm3[:, oth * 1024 : (oth + 1) * 1024], whalf, 1024,
                         f"uh{oth}")
            wm.append(wm3)

            # ---- main GEMM, transposed: psum[o, t] = wm.T @ x ----
            for g in range(G_PER_CORE):
                # prefetch next group's x before this group's out-DMAs can
                # block SP's issue FIFO on drain semaphores; hold each
                # prefetch back (modeled time) so it does not steal DMA
                # bandwidth from the startup-critical w + x(g0) transfers
                if g + 1 < G_PER_CORE:
                    for ts in range(4):
                        nc.sync.dma_start(
                            xgs[g + 1][:, ts * TOK_PER_GROUP : (ts + 1) * TOK_PER_GROUP],
                            x[(g + 1) * KT + ts],
                        )
                xg = xgs[g]
                for ot in range(OT):
                    oc = op.tile([128, TOK_PER_GROUP], BF16, tag="oc", name=f"oc{g}_{ot}")
                    units = []
                    for ts in range(4):
                        units.append(psp.tile([128, 512], F32, tag="ps", name=f"ps{g}_{ot}_{ts}"))
                    for kt in range(KT):
                        off = (ot // 2) * 1024 + kt * 256 + (ot % 2) * 128
                        wslice = wm[g][:, off : off + 128]
                        for ts in range(4):
                            nc.tensor.matmul(
                                units[ts][:],
                                wslice,
                                xg[:, ts * 2048 + kt * 512 : ts * 2048 + (kt + 1) * 512],
                                start=(kt == 0),
                                stop=(kt == KT - 1),
                            )
                    last = g == G_PER_CORE - 1
                    for ts in range(4):
                        mcol = mbv[:, ot * G_PER_CORE + g : ot * G_PER_CORE + g + 1]
                        if last and ts >= 2:
                            # DVE is idle once mixing ends; split the final
                            # group's drains to shorten the kernel tail
                            nc.vector.tensor_scalar(
                                oc[:, ts * 512 : (ts + 1) * 512],
                                units[ts][:], mcol, None, AluOpType.add,
                            )
                        else:
                            nc.scalar.add(
                                oc[:, ts * 512 : (ts + 1) * 512],
                                units[ts][:], mcol,
                            )
                        if last:
                            # fine-grained out-DMA on the final group to
                            # shorten the kernel tail
                            t0 = g * TOK_PER_GROUP + ts * 512
                            nc.sync.dma_start(
                                out[ot * 128 : (ot + 1) * 128, t0 : t0 + 512],
                                oc[:, ts * 512 : (ts + 1) * 512],
                            )
                    if not last:
                        nc.sync.dma_start(
                            out[ot * 128 : (ot + 1) * 128, g * TOK_PER_GROUP : (g + 1) * TOK_PER_GROUP],
                            oc[:],
                        )
    nc.finalize()
    return nc


def kernel(x, coefficients, weight_experts, bias_experts, weight_shared, bias_shared, sizes):
    x = np.asarray(x)
    coefficients = np.asarray(coefficients)
    weight_experts = np.asarray(weight_experts)
    bias_experts = np.asarray(bias_experts)
    weight_shared = np.asarray(weight_shared)
    bias_shared = np.asarray(bias_shared)

    if "nc" not in _CACHE:
        _CACHE["nc"] = _build()
    nc = _CACHE["nc"]

    # ---- host-side layout prep ----
    bf16 = ml_dtypes.bfloat16
    wt9 = np.empty((N_EXPERTS + 1, IN_F, OUT_F), np.float32)
    wt9[0] = weight_shared.T
    for e in range(N_EXPERTS):
        wt9[1 + e] = weight_experts[e].T
    # [piece][kt][p][oth][o256] -> [piece][p][oth][kt][o256]
    wt_np = np.ascontiguousarray(
        wt9.reshape(N_EXPERTS + 1, KT, 128, 2, 256).transpose(0, 2, 3, 1, 4)
    ).astype(bf16).reshape(N_EXPERTS + 1, 128, WSZ)
    ball_np = np.empty((N_EXPERTS + 1, OUT_F), np.float32)
    ball_np[:N_EXPERTS] = bias_experts
    ball_np[N_EXPERTS] = bias_shared

    in_maps = []
    for c in range(N_CORES):
        gs = slice(c * G_PER_CORE, (c + 1) * G_PER_CORE)
        cg = coefficients[gs]  # [4, 8]
        cb_np = np.broadcast_to(
            cg.reshape(1, -1), (128, G_PER_CORE * N_EXPERTS)
        ).copy()
        cx_np = np.empty((N_EXPERTS + 1, G_PER_CORE), np.float32)
        cx_np[:N_EXPERTS] = cg.T
        cx_np[N_EXPERTS] = 1.0
        xs = x[c * TOK_PER_CORE : (c + 1) * TOK_PER_CORE]
        # [g][ts][t512][kt][p] -> [g][ts][p][kt][t512]
        x_np = np.ascontiguousarray(
            xs.reshape(G_PER_CORE, 4, 512, KT, 128).transpose(0, 1, 4, 3, 2)
        ).astype(bf16).reshape(G_PER_CORE * 4, 128, TOK_PER_GROUP)
        in_maps.append(
            {
                "x": x_np,
                "wt": wt_np,
                "cb": cb_np,
                "cx": cx_np,
                "ball": ball_np,
            }
        )

    res = run_bass_kernel_spmd(nc, in_maps, core_ids=list(range(N_CORES)))
    return np.concatenate(
        [res.results[c]["out"].astype(np.float32).T for c in range(N_CORES)], axis=0
    )
